# revision 22
# baseline (speedup 1.0000x reference)
"""Trainium2 Bass kernel: 3x3 same-padding Conv2D, NCHW.

Input  (16, 64, 128, 128) f32, weights (128, 64, 3, 3) OIHW, bias (128,).
Output (16, 128, 128, 128) f32.

Strategy: data-parallel over batch — 2 images per NeuronCore on 8 cores.
Per core the conv runs as accumulated TensorEngine matmuls over
(C_in x tap) contractions:

  - The host pre-builds a padded dual fp16 layout per image,
    [128, 130*130]: partitions 0-63 (copy A) hold the zero-padded image
    shifted down one row (A[r] = padded row r-1), partitions 64-127
    (copy B) the padded rows directly (B[r] = padded row r).
  - Input DMA: chunk edges are 4096B-aligned per partition row so every
    DMA packet is a full 4096B (4420B rows split 4096+324 and halve DMA
    ring efficiency — measured ~245 GB/s vs ~390 GB/s).  Three small
    lead-in chunks let the first supergroup start ~1us earlier.  Chunks
    alternate between the sync and gpsimd HWDGE rings so descriptor
    generation (~0.6us per DMA) overlaps.
  - Supergroups of 16 output rows use four PSUM banks (bank j holds
    rows h+4j..h+4j+3; 4*128 = 512 f32 = one bank), two supergroups in
    flight (8 banks).  Per supergroup:
      K=128 phase: per kw, one w1[kw] load feeds 4 matmuls (taps
      (kh=0,kw) on A + (kh=1,kw) on B fused in one K=128 contraction).
      K=64 phase: the (kh=2,kw) taps are issued as ADJACENT matmuls on
      disjoint partition halves (banks 0/2 from A on partitions 0-63,
      banks 1/3 from B on 64-127) so the PE runs pairs concurrently.
    => 18 matmul slots per 16 rows, the K=128-packing ideal.
  - Epilogue: ScalarE and VectorE each bias-add two banks into a shared
    [128, 2048] tile; two 512KB store DMAs per supergroup, one on the
    scalar HWDGE ring and one on the vector ring (so stores never queue
    behind input chunks, and the final store is small -> short tail).

Operands are fp16 (cast host-side; 1 PE cycle/row + FWL weight loads,
rel err ~2.8e-4).  "f32r" mode (TF32-like) is kept as a fallback.
Every instruction may carry at most ONE semaphore wait on this
toolchain — bacc.Bacc's compile() pipeline (generate_event_semaphores)
enforces that, which is why this builds a Bacc, not a raw bass.Bass.
"""

import sys

if "/opt/trn_rl_repo" not in sys.path:
    sys.path.insert(0, "/opt/trn_rl_repo")

import numpy as np

N_CORES = 8
IMGS_PER_CORE = 2
H = 128
W = 128
CIN = 64
COUT = 128
WPAD = W + 2  # 130: one zero column each side
HPAD = H + 2  # 130 rows (pad row above and below)
ROWS_PER_BANK = 4  # 4*128 = 512 free elements = one PSUM bank
SG_ROWS = 32  # supergroup: all 8 PSUM banks
NBANKS = SG_ROWS // ROWS_PER_BANK  # 8

# "f32r": fp32 storage, TF32-like matmul (rel err ~2e-4, ~3 PE cycles/row)
# "f16": fp16 operands via host-side cast (rel err ~3e-4, 1 PE cycle/row)
DTYPE_MODE = "f16"

# Input chunk edges (elements of the flat [128, 130*130] image).  All
# interior edges are multiples of 2048 elems = 4096B, so every DMA
# descriptor row is whole 4096B packets.  Four small lead-in chunks
# (576 elems = 1152B = 1 packet) cover the first supergroup's rows
# (needs up to elem 17*130 = 2210 <= 2304) with minimum latency —
# alternating between two HW rings, the first matmul's data (<=517)
# lands ~1.4us sooner than with one 768-elem lead-in per ring.
X_EDGES = [0, 576, 1152, 1728, 2304] + list(range(4352, 16641, 2048)) + [16900]
# bank j of supergroup 0 needs flat elements < (4j+6)*130; lead-in
# chunk j (576 elems) covers bank j's reads, so banks unblock one by
# one as the two rings race the PE through supergroup 0.

_cache = {}


def _build_nc(mode=None):
    import concourse.mybir as mybir
    from concourse import bacc
    from concourse.tile import TileContext

    mode = mode or DTYPE_MODE
    f32 = mybir.dt.float32
    f32r = mybir.dt.float32r
    cdt = {"f32r": f32r, "f16": mybir.dt.float16}[mode]

    nc = bacc.Bacc(target_bir_lowering=False)
    x_d = nc.dram_tensor(
        "x", [IMGS_PER_CORE, 128, HPAD * WPAD], cdt, kind="ExternalInput"
    )
    # packed weights+bias:
    #   cols 0..383   : w1[t*64+ci, kw*128+co] = W[co, ci, t, kw], taps kh=t in {0,1}
    #   cols 384..767 : w2[ci, kw*128+co] = W[co, ci, 2, kw] (dup'd on rows 64-127)
    wb_d = nc.dram_tensor("wb", [128, 6 * COUT], cdt, kind="ExternalInput")
    b_d = nc.dram_tensor("b", [COUT, 1], f32, kind="ExternalInput")
    out_d = nc.dram_tensor(
        "out", [IMGS_PER_CORE, COUT, H, W], f32, kind="ExternalOutput"
    )

    with TileContext(nc) as tc:
        with (
            tc.tile_pool(name="wpool", bufs=1) as wpool,
            tc.tile_pool(name="xpool", bufs=2) as xpool,
            tc.tile_pool(name="opool", bufs=3) as opool,
            tc.tile_pool(name="pspool", bufs=1, space="PSUM") as pspool,
        ):
            wb_sb = wpool.tile([128, 6 * COUT], cdt)
            # split the weight load: the first LDWEIGHTS only needs
            # w1[kw=0] (cols 0:128, 32KB) — land it first on sync so the
            # first matmul isn't gated on the full 196KB packed tile
            nc.sync.dma_start(out=wb_sb[:, 0:COUT], in_=wb_d[:, 0:COUT])
            nc.scalar.dma_start(out=wb_sb[:, COUT:], in_=wb_d[:, COUT:])
            w1_sb = wb_sb[:, 0 : 3 * COUT]
            w2_sb = wb_sb[0:CIN, 3 * COUT : 6 * COUT]
            w2b_sb = wb_sb[CIN:128, 3 * COUT : 6 * COUT]
            b_f32 = wpool.tile([COUT, 1], f32)
            # bias on the gpsimd ring: keeps the sync ring free for the
            # first input chunks (bias isn't needed until the first
            # supergroup's epilogue, ~10us in)
            nc.gpsimd.dma_start(out=b_f32[:], in_=b_d[:])
            b_sb = b_f32[:]

            chunks = list(zip(X_EDGES[:-1], X_EDGES[1:]))
            n_chunks = len(chunks)
            # image-1 chunk issues are interleaved into image-0's
            # supergroup loop on the scalar engine, so they are paced by
            # compute progress (~1-2 chunks per 3.9us supergroup) instead
            # of flooding the shared DMA-engine pool and starving stores.
            n_sgs = H // SG_ROWS
            img1_sched = [[] for _ in range(n_sgs)]
            base, extra = divmod(n_chunks, n_sgs)
            ci = 0
            for s in range(n_sgs):
                take = base + (1 if s < extra else 0)
                img1_sched[s] = list(range(ci, ci + take))
                ci += take
            assert ci == n_chunks

            X0 = xpool.tile([128, HPAD * WPAD], cdt)
            X1 = xpool.tile([128, HPAD * WPAD], cdt)
            X_tiles = [X0, X1]
            # image-0 chunks split across the two fast HWDGE rings (sync
            # leads with c0 since scalar's queue head is the weight
            # tile); the gpsimd SW ring has multi-us per-DMA latency and
            # is only used for the tiny bias load.
            for k, (e0, e1) in enumerate(chunks):
                eng = nc.sync if k % 2 == 0 else nc.scalar
                eng.dma_start(out=X0[:, e0:e1], in_=x_d[0, :, e0:e1])

            for img in range(IMGS_PER_CORE):
                X = X_tiles[img]
                X3 = X.rearrange("p (r c) -> p r c", c=WPAD)

                for h in range(0, H, SG_ROWS):
                    s_idx = h // SG_ROWS
                    ps = [
                        pspool.tile(
                            [COUT, ROWS_PER_BANK * W],
                            f32,
                            tag=f"ps{j}",
                            name=f"ps{j}",
                        )
                        for j in range(NBANKS)
                    ]
                    # K=128 phase: taps (kh=0,kw) on A + (kh=1,kw) on B.
                    # One weight tile per kw feeds all eight banks, so the
                    # per-matmul LDWEIGHTS always hides under the previous
                    # matmul's streaming.
                    for kw in range(3):
                        for j in range(NBANKS):
                            r = h + ROWS_PER_BANK * j
                            nc.tensor.matmul(
                                ps[j][:],
                                w1_sb[:, kw * COUT : (kw + 1) * COUT],
                                X3[:, r : r + ROWS_PER_BANK, kw : kw + W],
                                start=(kw == 0),
                                stop=False,
                            )
                    # K=64 phase: (kh=2,kw) taps as concurrent pairs on
                    # disjoint partition halves (A-half -> even banks,
                    # B-half -> odd banks, different PSUM banks).
                    # Bank-pair-major, kw inner: pair (0,1) stops nine
                    # pair-slots before the supergroup ends, so its
                    # evacuation (and the next supergroup's use of those
                    # banks) overlaps the remaining pair-slots.
                    for j in range(0, NBANKS, 2):
                        for kw in range(3):
                            rA = h + ROWS_PER_BANK * j + 2
                            rB = h + ROWS_PER_BANK * (j + 1) + 1
                            nc.tensor.matmul(
                                ps[j][:],
                                w2_sb[:, kw * COUT : (kw + 1) * COUT],
                                X3[0:CIN, rA : rA + ROWS_PER_BANK, kw : kw + W],
                                start=False,
                                stop=(kw == 2),
                            )
                            nc.tensor.matmul(
                                ps[j + 1][:],
                                w2b_sb[:, kw * COUT : (kw + 1) * COUT],
                                X3[CIN:128, rB : rB + ROWS_PER_BANK, kw : kw + W],
                                start=False,
                                stop=(kw == 2),
                            )
                    # bias-add while evacuating PSUM into one 32-row tile;
                    # ScalarE takes even banks, VectorE odd banks, in bank
                    # order so early-stopping bank pairs evacuate while
                    # later pairs are still accumulating.
                    FB = ROWS_PER_BANK * W  # 512
                    ob = opool.tile([COUT, NBANKS * FB], f32)
                    for j in range(0, NBANKS, 2):
                        nc.scalar.add(
                            ob[:, j * FB : (j + 1) * FB], ps[j][:], b_sb
                        )
                        nc.vector.tensor_scalar_add(
                            ob[:, (j + 1) * FB : (j + 2) * FB], ps[j + 1][:], b_sb
                        )
                    ob3 = ob.rearrange("p (r c) -> p r c", c=W)
                    last_sg = img == IMGS_PER_CORE - 1 and h == H - SG_ROWS
                    if last_sg:
                        # final supergroup: 256KB quarter-stores issued
                        # per bank-pair as evacuations land, alternating
                        # rings, so the post-compute drain is short
                        for q in range(NBANKS):
                            eng2 = nc.scalar if q % 2 == 0 else nc.sync
                            eng2.dma_start(
                                out=out_d[img, :, h + 4 * q : h + 4 * (q + 1), :],
                                in_=ob3[:, 4 * q : 4 * (q + 1)],
                            )
                    else:
                        # four 512KB half-stores per supergroup, one per
                        # bank pair, issued as soon as that pair is
                        # evacuated; alternating scalar/sync rings except
                        # supergroup 0 (sync still streams input then)
                        for q in range(0, NBANKS, 2):
                            on_scalar = (q // 2) % 2 == 0 or (
                                img == 0 and s_idx == 0
                            )
                            eng2 = nc.scalar if on_scalar else nc.sync
                            eng2.dma_start(
                                out=out_d[img, :, h + 4 * q : h + 4 * (q + 2), :],
                                in_=ob3[:, 4 * q : 4 * (q + 2)],
                            )
                    if img == 0:
                        # compute-paced image-1 input: issued by the
                        # scalar engine after this supergroup's stores
                        for k in img1_sched[s_idx]:
                            e0, e1 = chunks[k]
                            nc.scalar.dma_start(
                                out=X1[:, e0:e1], in_=x_d[1, :, e0:e1]
                            )
    nc.compile()
    return nc


def _get_nc(mode=None):
    mode = mode or DTYPE_MODE
    if mode not in _cache:
        _cache[mode] = _build_nc(mode)
    return _cache[mode]


def _make_dual(images):
    """images: [n, 64, 128, 128] -> [n, 128, HPAD*WPAD] dual padded layout."""
    n = images.shape[0]
    zp = np.zeros((n, CIN, HPAD, WPAD), dtype=np.float32)
    zp[:, :, 1 : H + 1, 1 : W + 1] = images  # padded rows 0..129
    dual = np.empty((n, 128, HPAD, WPAD), dtype=np.float32)
    dual[:, 0:CIN] = zp  # A[r] = padded row r-1 shape-wise (row r of zp)
    dual[:, CIN:128, 0 : HPAD - 1] = zp[:, :, 1:HPAD]  # B[r] = padded row r
    dual[:, CIN:128, HPAD - 1] = 0.0  # B row 129 unread
    return np.ascontiguousarray(dual.reshape(n, 128, HPAD * WPAD))


def _prepare_in_maps(input_tensor, weights, bias, mode=None):
    mode = mode or DTYPE_MODE
    hdt = np.float32 if mode == "f32r" else np.float16
    input_tensor = np.asarray(input_tensor, dtype=np.float32)
    weights = np.asarray(weights, dtype=np.float32)
    bias = np.asarray(bias, dtype=np.float32)
    wb = np.zeros((128, 6 * COUT), dtype=np.float32)
    # [co, ci, kh, kw] -> w1[t*64+ci, kw*128+co], w2[ci, kw*128+co]
    wb[:, 0 : 3 * COUT] = (
        weights[:, :, 0:2, :].transpose(2, 1, 3, 0).reshape(128, 3 * COUT)
    )
    w2 = weights[:, :, 2, :].transpose(1, 2, 0).reshape(CIN, 3 * COUT)
    wb[0:CIN, 3 * COUT : 6 * COUT] = w2
    wb[CIN:128, 3 * COUT : 6 * COUT] = w2  # duplicate for partition-64 row tiles
    wb = np.ascontiguousarray(wb.astype(hdt))
    b = np.ascontiguousarray(bias.reshape(COUT, 1))
    in_maps = []
    for c in range(N_CORES):
        shard = _make_dual(
            input_tensor[c * IMGS_PER_CORE : (c + 1) * IMGS_PER_CORE]
        ).astype(hdt)
        in_maps.append({"x": shard, "wb": wb, "b": b})
    return in_maps


def _gather(results):
    return np.concatenate([results[c]["out"] for c in range(N_CORES)], axis=0)


def kernel(input_tensor, weights, bias):
    from concourse.bass_utils import run_bass_kernel_spmd

    nc = _get_nc()
    in_maps = _prepare_in_maps(input_tensor, weights, bias)
    res = run_bass_kernel_spmd(nc, in_maps, core_ids=list(range(N_CORES)))
    return _gather(res.results)


# revision 25
# speedup vs baseline: 1.0202x; 1.0202x over previous
"""Trainium2 Bass kernel: 3x3 same-padding Conv2D, NCHW.

Input  (16, 64, 128, 128) f32, weights (128, 64, 3, 3) OIHW, bias (128,).
Output (16, 128, 128, 128) f32.

Strategy: data-parallel over batch — 2 images per NeuronCore on 8 cores.
Per core the conv runs as accumulated TensorEngine matmuls over
(C_in x tap) contractions:

  - The host pre-builds a padded dual fp16 layout per image,
    [128, 130*130]: partitions 0-63 (copy A) hold the zero-padded image
    shifted down one row (A[r] = padded row r-1), partitions 64-127
    (copy B) the padded rows directly (B[r] = padded row r).
  - Input DMA: chunk edges are 4096B-aligned per partition row so every
    DMA packet is a full 4096B (4420B rows split 4096+324 and halve DMA
    ring efficiency — measured ~245 GB/s vs ~390 GB/s).  Three small
    lead-in chunks let the first supergroup start ~1us earlier.  Chunks
    alternate between the sync and gpsimd HWDGE rings so descriptor
    generation (~0.6us per DMA) overlaps.
  - Supergroups of 16 output rows use four PSUM banks (bank j holds
    rows h+4j..h+4j+3; 4*128 = 512 f32 = one bank), two supergroups in
    flight (8 banks).  Per supergroup:
      K=128 phase: per kw, one w1[kw] load feeds 4 matmuls (taps
      (kh=0,kw) on A + (kh=1,kw) on B fused in one K=128 contraction).
      K=64 phase: the (kh=2,kw) taps are issued as ADJACENT matmuls on
      disjoint partition halves (banks 0/2 from A on partitions 0-63,
      banks 1/3 from B on 64-127) so the PE runs pairs concurrently.
    => 18 matmul slots per 16 rows, the K=128-packing ideal.
  - Epilogue: ScalarE and VectorE each bias-add two banks into a shared
    [128, 2048] tile; two 512KB store DMAs per supergroup, one on the
    scalar HWDGE ring and one on the vector ring (so stores never queue
    behind input chunks, and the final store is small -> short tail).

Operands are fp16 (cast host-side; 1 PE cycle/row + FWL weight loads,
rel err ~2.8e-4).  "f32r" mode (TF32-like) is kept as a fallback.
Every instruction may carry at most ONE semaphore wait on this
toolchain — bacc.Bacc's compile() pipeline (generate_event_semaphores)
enforces that, which is why this builds a Bacc, not a raw bass.Bass.
"""

import sys

if "/opt/trn_rl_repo" not in sys.path:
    sys.path.insert(0, "/opt/trn_rl_repo")

import numpy as np

N_CORES = 8
IMGS_PER_CORE = 2
H = 128
W = 128
CIN = 64
COUT = 128
WPAD = W + 2  # 130: one zero column each side
HPAD = H + 2  # 130 rows (pad row above and below)
ROWS_PER_BANK = 4  # 4*128 = 512 free elements = one PSUM bank
SG_ROWS = 16  # supergroup: 4 PSUM banks

# "f32r": fp32 storage, TF32-like matmul (rel err ~2e-4, ~3 PE cycles/row)
# "f16": fp16 operands via host-side cast (rel err ~3e-4, 1 PE cycle/row)
DTYPE_MODE = "f16"

# Input chunk edges (elements of the flat [128, 130*130] image).  All
# interior edges are multiples of 2048 elems = 4096B, so every DMA
# descriptor row is whole 4096B packets.  Four small lead-in chunks
# (576 elems = 1152B = 1 packet) cover the first supergroup's rows
# (needs up to elem 17*130 = 2210 <= 2304) with minimum latency —
# alternating between two HW rings, the first matmul's data (<=517)
# lands ~1.4us sooner than with one 768-elem lead-in per ring.
X_EDGES = [0, 576, 1152, 1728, 2304] + list(range(4352, 16641, 2048)) + [16900]
# bank j of supergroup 0 needs flat elements < (4j+6)*130; lead-in
# chunk j (576 elems) covers bank j's reads, so banks unblock one by
# one as the two rings race the PE through supergroup 0.

_cache = {}


def _build_nc(mode=None):
    import concourse.mybir as mybir
    from concourse import bacc
    from concourse.tile import TileContext

    mode = mode or DTYPE_MODE
    f32 = mybir.dt.float32
    f32r = mybir.dt.float32r
    cdt = {"f32r": f32r, "f16": mybir.dt.float16}[mode]

    nc = bacc.Bacc(target_bir_lowering=False)
    x_d = nc.dram_tensor(
        "x", [IMGS_PER_CORE, 128, HPAD * WPAD], cdt, kind="ExternalInput"
    )
    # packed weights+bias:
    #   cols 0..383   : w1[t*64+ci, kw*128+co] = W[co, ci, t, kw], taps kh=t in {0,1}
    #   cols 384..767 : w2[ci, kw*128+co] = W[co, ci, 2, kw] (dup'd on rows 64-127)
    wb_d = nc.dram_tensor("wb", [128, 6 * COUT], cdt, kind="ExternalInput")
    b_d = nc.dram_tensor("b", [COUT, 1], f32, kind="ExternalInput")
    out_d = nc.dram_tensor(
        "out", [IMGS_PER_CORE, COUT, H, W], f32, kind="ExternalOutput"
    )

    with TileContext(nc) as tc:
        with (
            tc.tile_pool(name="wpool", bufs=1) as wpool,
            tc.tile_pool(name="xpool", bufs=2) as xpool,
            tc.tile_pool(name="opool", bufs=6) as opool,
            tc.tile_pool(name="pspool", bufs=2, space="PSUM") as pspool,
        ):
            wb_sb = wpool.tile([128, 6 * COUT], cdt)
            # split the weight load: the first LDWEIGHTS only needs
            # w1[kw=0] (cols 0:128, 32KB) — land it first on sync so the
            # first matmul isn't gated on the full 196KB packed tile
            nc.sync.dma_start(out=wb_sb[:, 0:COUT], in_=wb_d[:, 0:COUT])
            nc.scalar.dma_start(out=wb_sb[:, COUT:], in_=wb_d[:, COUT:])
            w1_sb = wb_sb[:, 0 : 3 * COUT]
            w2_sb = wb_sb[0:CIN, 3 * COUT : 6 * COUT]
            w2b_sb = wb_sb[CIN:128, 3 * COUT : 6 * COUT]
            b_f32 = wpool.tile([COUT, 1], f32)
            # bias on the gpsimd ring: keeps the sync ring free for the
            # first input chunks (bias isn't needed until the first
            # supergroup's epilogue, ~10us in)
            nc.gpsimd.dma_start(out=b_f32[:], in_=b_d[:])
            b_sb = b_f32[:]

            chunks = list(zip(X_EDGES[:-1], X_EDGES[1:]))
            n_chunks = len(chunks)
            # image-1 chunk issues are interleaved into image-0's
            # supergroup loop on the scalar engine, so they are paced by
            # compute progress (~1-2 chunks per 3.9us supergroup) instead
            # of flooding the shared DMA-engine pool and starving stores.
            n_sgs = H // SG_ROWS
            img1_sched = [[] for _ in range(n_sgs)]
            ci = 0
            for s in range(n_sgs):
                take = 2 if s < n_chunks - n_sgs else 1
                img1_sched[s] = list(range(ci, min(ci + take, n_chunks)))
                ci += take

            X0 = xpool.tile([128, HPAD * WPAD], cdt)
            X1 = xpool.tile([128, HPAD * WPAD], cdt)
            X_tiles = [X0, X1]
            # image-0 chunks split across the two fast HWDGE rings, with
            # two mid chunks (c3, c5, needed ~11-20us) offloaded to the
            # gpsimd SW ring (high per-DMA latency but it adds a third
            # ~150GB/s stream during the startup crunch when all eight
            # cores are loading inputs simultaneously)
            for k, (e0, e1) in enumerate(chunks):
                if k in (3, 5):
                    eng = nc.gpsimd
                elif k % 2 == 0:
                    eng = nc.sync
                else:
                    eng = nc.scalar
                eng.dma_start(out=X0[:, e0:e1], in_=x_d[0, :, e0:e1])

            for img in range(IMGS_PER_CORE):
                X = X_tiles[img]
                X3 = X.rearrange("p (r c) -> p r c", c=WPAD)

                for h in range(0, H, SG_ROWS):
                    s_idx = h // SG_ROWS
                    ps = [
                        pspool.tile(
                            [COUT, ROWS_PER_BANK * W],
                            f32,
                            tag=f"ps{j}",
                            name=f"ps{j}",
                        )
                        for j in range(4)
                    ]
                    # K=128 phase: taps (kh=0,kw) on A + (kh=1,kw) on B.
                    # One weight tile per kw feeds all four banks, so the
                    # per-matmul LDWEIGHTS always hides under the previous
                    # matmul's streaming.
                    for kw in range(3):
                        for j in range(4):
                            r = h + ROWS_PER_BANK * j
                            nc.tensor.matmul(
                                ps[j][:],
                                w1_sb[:, kw * COUT : (kw + 1) * COUT],
                                X3[:, r : r + ROWS_PER_BANK, kw : kw + W],
                                start=(kw == 0),
                                stop=False,
                            )
                    # K=64 phase: (kh=2,kw) taps as concurrent pairs on
                    # disjoint partition halves (A-half -> even banks,
                    # B-half -> odd banks, different PSUM banks).
                    for kw in range(3):
                        for j in (0, 2):
                            rA = h + ROWS_PER_BANK * j + 2
                            rB = h + ROWS_PER_BANK * (j + 1) + 1
                            nc.tensor.matmul(
                                ps[j][:],
                                w2_sb[:, kw * COUT : (kw + 1) * COUT],
                                X3[0:CIN, rA : rA + ROWS_PER_BANK, kw : kw + W],
                                start=False,
                                stop=(kw == 2),
                            )
                            nc.tensor.matmul(
                                ps[j + 1][:],
                                w2b_sb[:, kw * COUT : (kw + 1) * COUT],
                                X3[CIN:128, rB : rB + ROWS_PER_BANK, kw : kw + W],
                                start=False,
                                stop=(kw == 2),
                            )
                    # bias-add while evacuating PSUM into one 16-row tile;
                    # ScalarE takes banks 0,2 and VectorE banks 1,3.  Two
                    # 512KB store DMAs per supergroup on the scalar and
                    # vector HWDGE rings (stores never queue behind input
                    # chunks on sync/gpsimd).
                    FB = ROWS_PER_BANK * W  # 512
                    ob = opool.tile([COUT, 4 * FB], f32)
                    last_sg = img == IMGS_PER_CORE - 1 and h == H - SG_ROWS
                    if last_sg:
                        # final supergroup: banks 0/1 stop one pair-slot
                        # before banks 2/3, so give each engine one early
                        # and one late bank — scalar runs b0 then b3,
                        # vector b1 then b2; all four banks are
                        # evacuated ~1.4us after the last matmul
                        nc.scalar.add(ob[:, 0:FB], ps[0][:], b_sb)
                        nc.scalar.add(ob[:, 3 * FB : 4 * FB], ps[3][:], b_sb)
                        nc.vector.tensor_scalar_add(
                            ob[:, FB : 2 * FB], ps[1][:], b_sb
                        )
                        nc.vector.tensor_scalar_add(
                            ob[:, 2 * FB : 3 * FB], ps[2][:], b_sb
                        )
                    else:
                        nc.scalar.add(ob[:, 0:FB], ps[0][:], b_sb)
                        nc.vector.tensor_scalar_add(
                            ob[:, FB : 2 * FB], ps[1][:], b_sb
                        )
                        nc.scalar.add(ob[:, 2 * FB : 3 * FB], ps[2][:], b_sb)
                        nc.vector.tensor_scalar_add(
                            ob[:, 3 * FB : 4 * FB], ps[3][:], b_sb
                        )
                    ob3 = ob.rearrange("p (r c) -> p r c", c=W)
                    if img == IMGS_PER_CORE - 1 and h >= H - 2 * SG_ROWS:
                        # final two supergroups: each bank's 256KB store
                        # is split into two 128KB partition-halves, one
                        # per ring (desc rows stay 4096B), so both queues
                        # drain every bank in parallel and the post-
                        # compute tail is short
                        for q in range(4):
                            nc.scalar.dma_start(
                                out=out_d[
                                    img, 0:64, h + 4 * q : h + 4 * (q + 1), :
                                ],
                                in_=ob3[0:64, 4 * q : 4 * (q + 1)],
                            )
                            nc.sync.dma_start(
                                out=out_d[
                                    img, 64:128, h + 4 * q : h + 4 * (q + 1), :
                                ],
                                in_=ob3[64:128, 4 * q : 4 * (q + 1)],
                            )
                    else:
                        nc.scalar.dma_start(
                            out=out_d[img, :, h : h + 8, :], in_=ob3[:, 0:8]
                        )
                        # second half-store on the sync ring once image
                        # 0's input chunks have drained off it (~21us);
                        # before that, both halves go on scalar
                        eng2 = nc.scalar if (img == 0 and s_idx < 2) else nc.sync
                        eng2.dma_start(
                            out=out_d[img, :, h + 8 : h + 16, :], in_=ob3[:, 8:16]
                        )
                    if img == 0:
                        # compute-paced image-1 input: issued by the
                        # scalar engine after this supergroup's stores
                        for k in img1_sched[s_idx]:
                            e0, e1 = chunks[k]
                            nc.scalar.dma_start(
                                out=X1[:, e0:e1], in_=x_d[1, :, e0:e1]
                            )
    nc.compile()
    return nc


def _get_nc(mode=None):
    mode = mode or DTYPE_MODE
    if mode not in _cache:
        _cache[mode] = _build_nc(mode)
    return _cache[mode]


def _make_dual(images):
    """images: [n, 64, 128, 128] -> [n, 128, HPAD*WPAD] dual padded layout."""
    n = images.shape[0]
    zp = np.zeros((n, CIN, HPAD, WPAD), dtype=np.float32)
    zp[:, :, 1 : H + 1, 1 : W + 1] = images  # padded rows 0..129
    dual = np.empty((n, 128, HPAD, WPAD), dtype=np.float32)
    dual[:, 0:CIN] = zp  # A[r] = padded row r-1 shape-wise (row r of zp)
    dual[:, CIN:128, 0 : HPAD - 1] = zp[:, :, 1:HPAD]  # B[r] = padded row r
    dual[:, CIN:128, HPAD - 1] = 0.0  # B row 129 unread
    return np.ascontiguousarray(dual.reshape(n, 128, HPAD * WPAD))


def _prepare_in_maps(input_tensor, weights, bias, mode=None):
    mode = mode or DTYPE_MODE
    hdt = np.float32 if mode == "f32r" else np.float16
    input_tensor = np.asarray(input_tensor, dtype=np.float32)
    weights = np.asarray(weights, dtype=np.float32)
    bias = np.asarray(bias, dtype=np.float32)
    wb = np.zeros((128, 6 * COUT), dtype=np.float32)
    # [co, ci, kh, kw] -> w1[t*64+ci, kw*128+co], w2[ci, kw*128+co]
    wb[:, 0 : 3 * COUT] = (
        weights[:, :, 0:2, :].transpose(2, 1, 3, 0).reshape(128, 3 * COUT)
    )
    w2 = weights[:, :, 2, :].transpose(1, 2, 0).reshape(CIN, 3 * COUT)
    wb[0:CIN, 3 * COUT : 6 * COUT] = w2
    wb[CIN:128, 3 * COUT : 6 * COUT] = w2  # duplicate for partition-64 row tiles
    wb = np.ascontiguousarray(wb.astype(hdt))
    b = np.ascontiguousarray(bias.reshape(COUT, 1))
    in_maps = []
    for c in range(N_CORES):
        shard = _make_dual(
            input_tensor[c * IMGS_PER_CORE : (c + 1) * IMGS_PER_CORE]
        ).astype(hdt)
        in_maps.append({"x": shard, "wb": wb, "b": b})
    return in_maps


def _gather(results):
    return np.concatenate([results[c]["out"] for c in range(N_CORES)], axis=0)


def kernel(input_tensor, weights, bias):
    from concourse.bass_utils import run_bass_kernel_spmd

    nc = _get_nc()
    in_maps = _prepare_in_maps(input_tensor, weights, bias)
    res = run_bass_kernel_spmd(nc, in_maps, core_ids=list(range(N_CORES)))
    return _gather(res.results)


# revision 28
# speedup vs baseline: 1.0361x; 1.0156x over previous
"""Trainium2 Bass kernel: 3x3 same-padding Conv2D, NCHW.

Input  (16, 64, 128, 128) f32, weights (128, 64, 3, 3) OIHW, bias (128,).
Output (16, 128, 128, 128) f32.

Strategy: data-parallel over batch — 2 images per NeuronCore on 8 cores.
Per core the conv runs as accumulated TensorEngine matmuls over
(C_in x tap) contractions:

  - The host pre-builds a padded dual fp16 layout per image,
    [128, 130*130]: partitions 0-63 (copy A) hold the zero-padded image
    shifted down one row (A[r] = padded row r-1), partitions 64-127
    (copy B) the padded rows directly (B[r] = padded row r).
  - Input DMA: chunk edges are 4096B-aligned per partition row so every
    DMA packet is a full 4096B (4420B rows split 4096+324 and halve DMA
    ring efficiency — measured ~245 GB/s vs ~390 GB/s).  Three small
    lead-in chunks let the first supergroup start ~1us earlier.  Chunks
    alternate between the sync and gpsimd HWDGE rings so descriptor
    generation (~0.6us per DMA) overlaps.
  - Supergroups of 16 output rows use four PSUM banks (bank j holds
    rows h+4j..h+4j+3; 4*128 = 512 f32 = one bank), two supergroups in
    flight (8 banks).  Per supergroup:
      K=128 phase: per kw, one w1[kw] load feeds 4 matmuls (taps
      (kh=0,kw) on A + (kh=1,kw) on B fused in one K=128 contraction).
      K=64 phase: the (kh=2,kw) taps are issued as ADJACENT matmuls on
      disjoint partition halves (banks 0/2 from A on partitions 0-63,
      banks 1/3 from B on 64-127) so the PE runs pairs concurrently.
    => 18 matmul slots per 16 rows, the K=128-packing ideal.
  - Epilogue: ScalarE and VectorE each bias-add two banks into a shared
    [128, 2048] tile; two 512KB store DMAs per supergroup, one on the
    scalar HWDGE ring and one on the vector ring (so stores never queue
    behind input chunks, and the final store is small -> short tail).

Operands are fp16 (cast host-side; 1 PE cycle/row + FWL weight loads,
rel err ~2.8e-4).  "f32r" mode (TF32-like) is kept as a fallback.
Every instruction may carry at most ONE semaphore wait on this
toolchain — bacc.Bacc's compile() pipeline (generate_event_semaphores)
enforces that, which is why this builds a Bacc, not a raw bass.Bass.
"""

import sys

if "/opt/trn_rl_repo" not in sys.path:
    sys.path.insert(0, "/opt/trn_rl_repo")

import numpy as np

N_CORES = 8
IMGS_PER_CORE = 2
H = 128
W = 128
CIN = 64
COUT = 128
WPAD = W + 2  # 130: one zero column each side
HPAD = H + 2  # 130 rows (pad row above and below)
ROWS_PER_BANK = 4  # 4*128 = 512 free elements = one PSUM bank
SG_ROWS = 16  # supergroup: 4 PSUM banks

# "f32r": fp32 storage, TF32-like matmul (rel err ~2e-4, ~3 PE cycles/row)
# "f16": fp16 operands via host-side cast (rel err ~3e-4, 1 PE cycle/row)
DTYPE_MODE = "f16"

# Input chunk edges (elements of the flat [128, 130*130] image).  All
# interior edges are multiples of 2048 elems = 4096B, so every DMA
# descriptor row is whole 4096B packets.  Four small lead-in chunks
# (576 elems = 1152B = 1 packet) cover the first supergroup's rows
# (needs up to elem 17*130 = 2210 <= 2304) with minimum latency —
# alternating between two HW rings, the first matmul's data (<=517)
# lands ~1.4us sooner than with one 768-elem lead-in per ring.
X_EDGES = [0, 576, 1152, 1728, 2304] + list(range(4352, 16641, 2048)) + [16900]
# bank j of supergroup 0 needs flat elements < (4j+6)*130; lead-in
# chunk j (576 elems) covers bank j's reads, so banks unblock one by
# one as the two rings race the PE through supergroup 0.

_cache = {}


def _build_nc(mode=None):
    import concourse.mybir as mybir
    from concourse import bacc
    from concourse.tile import TileContext

    mode = mode or DTYPE_MODE
    f32 = mybir.dt.float32
    f32r = mybir.dt.float32r
    cdt = {"f32r": f32r, "f16": mybir.dt.float16}[mode]

    nc = bacc.Bacc(target_bir_lowering=False)
    x_d = nc.dram_tensor(
        "x", [IMGS_PER_CORE, 128, HPAD * WPAD], cdt, kind="ExternalInput"
    )
    # packed weights+bias:
    #   cols 0..383   : w1[t*64+ci, kw*128+co] = W[co, ci, t, kw], taps kh=t in {0,1}
    #   cols 384..767 : w2[ci, kw*128+co] = W[co, ci, 2, kw] (dup'd on rows 64-127)
    wb_d = nc.dram_tensor("wb", [128, 6 * COUT], cdt, kind="ExternalInput")
    b_d = nc.dram_tensor("b", [COUT, 1], f32, kind="ExternalInput")
    out_d = nc.dram_tensor(
        "out", [IMGS_PER_CORE, COUT, H, W], f32, kind="ExternalOutput"
    )

    with TileContext(nc) as tc:
        with (
            tc.tile_pool(name="wpool", bufs=1) as wpool,
            tc.tile_pool(name="xpool", bufs=2) as xpool,
            tc.tile_pool(name="opool", bufs=6) as opool,
            tc.tile_pool(name="pspool", bufs=2, space="PSUM") as pspool,
        ):
            wb_sb = wpool.tile([128, 6 * COUT], cdt)
            # split the weight load: the first LDWEIGHTS only needs
            # w1[kw=0] (cols 0:128, 32KB) — land it first on sync so the
            # first matmul isn't gated on the full 196KB packed tile
            nc.sync.dma_start(out=wb_sb[:, 0:COUT], in_=wb_d[:, 0:COUT])
            nc.scalar.dma_start(out=wb_sb[:, COUT:], in_=wb_d[:, COUT:])
            w1_sb = wb_sb[:, 0 : 3 * COUT]
            w2_sb = wb_sb[0:CIN, 3 * COUT : 6 * COUT]
            w2b_sb = wb_sb[CIN:128, 3 * COUT : 6 * COUT]
            b_f32 = wpool.tile([COUT, 1], f32)
            # bias on the gpsimd ring: keeps the sync ring free for the
            # first input chunks (bias isn't needed until the first
            # supergroup's epilogue, ~10us in)
            nc.gpsimd.dma_start(out=b_f32[:], in_=b_d[:])
            b_sb = b_f32[:]

            chunks = list(zip(X_EDGES[:-1], X_EDGES[1:]))
            n_chunks = len(chunks)
            # image-1 chunk issues are interleaved into image-0's
            # supergroup loop on the scalar engine, so they are paced by
            # compute progress (~1-2 chunks per 3.9us supergroup) instead
            # of flooding the shared DMA-engine pool and starving stores.
            n_sgs = H // SG_ROWS
            img1_sched = [[] for _ in range(n_sgs)]
            ci = 0
            for s in range(n_sgs):
                take = 2 if s < n_chunks - n_sgs else 1
                img1_sched[s] = list(range(ci, min(ci + take, n_chunks)))
                ci += take

            X0 = xpool.tile([128, HPAD * WPAD], cdt)
            X1 = xpool.tile([128, HPAD * WPAD], cdt)
            X_tiles = [X0, X1]
            # image-0 chunks split across the two fast HWDGE rings (sync
            # leads with c0 since scalar's queue head is the weight
            # tile); the gpsimd SW ring has multi-us per-DMA latency and
            # is only used for the tiny bias load.
            for k, (e0, e1) in enumerate(chunks):
                eng = nc.sync if k % 2 == 0 else nc.scalar
                eng.dma_start(out=X0[:, e0:e1], in_=x_d[0, :, e0:e1])

            for img in range(IMGS_PER_CORE):
                X = X_tiles[img]
                X3 = X.rearrange("p (r c) -> p r c", c=WPAD)

                for h in range(0, H, SG_ROWS):
                    s_idx = h // SG_ROWS
                    ps = [
                        pspool.tile(
                            [COUT, ROWS_PER_BANK * W],
                            f32,
                            tag=f"ps{j}",
                            name=f"ps{j}",
                        )
                        for j in range(4)
                    ]
                    # K=128 phase: taps (kh=0,kw) on A + (kh=1,kw) on B.
                    # One weight tile per kw feeds all four banks, so the
                    # per-matmul LDWEIGHTS always hides under the previous
                    # matmul's streaming.
                    for kw in range(3):
                        for j in range(4):
                            r = h + ROWS_PER_BANK * j
                            nc.tensor.matmul(
                                ps[j][:],
                                w1_sb[:, kw * COUT : (kw + 1) * COUT],
                                X3[:, r : r + ROWS_PER_BANK, kw : kw + W],
                                start=(kw == 0),
                                stop=False,
                            )
                    # K=64 phase: (kh=2,kw) taps as concurrent pairs on
                    # disjoint partition halves (A-half -> even banks,
                    # B-half -> odd banks, different PSUM banks).
                    for kw in range(3):
                        for j in (0, 2):
                            rA = h + ROWS_PER_BANK * j + 2
                            rB = h + ROWS_PER_BANK * (j + 1) + 1
                            nc.tensor.matmul(
                                ps[j][:],
                                w2_sb[:, kw * COUT : (kw + 1) * COUT],
                                X3[0:CIN, rA : rA + ROWS_PER_BANK, kw : kw + W],
                                start=False,
                                stop=(kw == 2),
                            )
                            nc.tensor.matmul(
                                ps[j + 1][:],
                                w2b_sb[:, kw * COUT : (kw + 1) * COUT],
                                X3[CIN:128, rB : rB + ROWS_PER_BANK, kw : kw + W],
                                start=False,
                                stop=(kw == 2),
                            )
                    # bias-add while evacuating PSUM into one 16-row tile;
                    # ScalarE takes banks 0,2 and VectorE banks 1,3.  Two
                    # 512KB store DMAs per supergroup on the scalar and
                    # vector HWDGE rings (stores never queue behind input
                    # chunks on sync/gpsimd).
                    FB = ROWS_PER_BANK * W  # 512
                    ob = opool.tile([COUT, 4 * FB], f32)
                    nc.scalar.add(ob[:, 0:FB], ps[0][:], b_sb)
                    nc.vector.tensor_scalar_add(ob[:, FB : 2 * FB], ps[1][:], b_sb)
                    nc.scalar.add(ob[:, 2 * FB : 3 * FB], ps[2][:], b_sb)
                    nc.vector.tensor_scalar_add(
                        ob[:, 3 * FB : 4 * FB], ps[3][:], b_sb
                    )
                    ob3 = ob.rearrange("p (r c) -> p r c", c=W)
                    # two 512KB half-stores per supergroup — per-queue
                    # DMA processing is serialized with ~1us fixed cost
                    # per DMA, so fewer/larger stores drain faster than
                    # fine-grained splits (measured).  Banks 0/1 stop one
                    # pair-slot early, so the first half-store issues
                    # ~0.4us after the supergroup's last matmul.
                    nc.scalar.dma_start(
                        out=out_d[img, :, h : h + 8, :], in_=ob3[:, 0:8]
                    )
                    # second half-store on the sync ring once image 0's
                    # input chunks have drained off it (~21us); before
                    # that, both halves go on scalar
                    eng2 = nc.scalar if (img == 0 and s_idx < 2) else nc.sync
                    eng2.dma_start(
                        out=out_d[img, :, h + 8 : h + 16, :], in_=ob3[:, 8:16]
                    )
                    if img == 0:
                        # compute-paced image-1 input: issued by the
                        # scalar engine after this supergroup's stores
                        for k in img1_sched[s_idx]:
                            e0, e1 = chunks[k]
                            nc.scalar.dma_start(
                                out=X1[:, e0:e1], in_=x_d[1, :, e0:e1]
                            )
    nc.compile()
    return nc


def _get_nc(mode=None):
    mode = mode or DTYPE_MODE
    if mode not in _cache:
        _cache[mode] = _build_nc(mode)
    return _cache[mode]


def _make_dual(images):
    """images: [n, 64, 128, 128] -> [n, 128, HPAD*WPAD] dual padded layout."""
    n = images.shape[0]
    zp = np.zeros((n, CIN, HPAD, WPAD), dtype=np.float32)
    zp[:, :, 1 : H + 1, 1 : W + 1] = images  # padded rows 0..129
    dual = np.empty((n, 128, HPAD, WPAD), dtype=np.float32)
    dual[:, 0:CIN] = zp  # A[r] = padded row r-1 shape-wise (row r of zp)
    dual[:, CIN:128, 0 : HPAD - 1] = zp[:, :, 1:HPAD]  # B[r] = padded row r
    dual[:, CIN:128, HPAD - 1] = 0.0  # B row 129 unread
    return np.ascontiguousarray(dual.reshape(n, 128, HPAD * WPAD))


def _prepare_in_maps(input_tensor, weights, bias, mode=None):
    mode = mode or DTYPE_MODE
    hdt = np.float32 if mode == "f32r" else np.float16
    input_tensor = np.asarray(input_tensor, dtype=np.float32)
    weights = np.asarray(weights, dtype=np.float32)
    bias = np.asarray(bias, dtype=np.float32)
    wb = np.zeros((128, 6 * COUT), dtype=np.float32)
    # [co, ci, kh, kw] -> w1[t*64+ci, kw*128+co], w2[ci, kw*128+co]
    wb[:, 0 : 3 * COUT] = (
        weights[:, :, 0:2, :].transpose(2, 1, 3, 0).reshape(128, 3 * COUT)
    )
    w2 = weights[:, :, 2, :].transpose(1, 2, 0).reshape(CIN, 3 * COUT)
    wb[0:CIN, 3 * COUT : 6 * COUT] = w2
    wb[CIN:128, 3 * COUT : 6 * COUT] = w2  # duplicate for partition-64 row tiles
    wb = np.ascontiguousarray(wb.astype(hdt))
    b = np.ascontiguousarray(bias.reshape(COUT, 1))
    in_maps = []
    for c in range(N_CORES):
        shard = _make_dual(
            input_tensor[c * IMGS_PER_CORE : (c + 1) * IMGS_PER_CORE]
        ).astype(hdt)
        in_maps.append({"x": shard, "wb": wb, "b": b})
    return in_maps


def _gather(results):
    return np.concatenate([results[c]["out"] for c in range(N_CORES)], axis=0)


def kernel(input_tensor, weights, bias):
    from concourse.bass_utils import run_bass_kernel_spmd

    nc = _get_nc()
    in_maps = _prepare_in_maps(input_tensor, weights, bias)
    res = run_bass_kernel_spmd(nc, in_maps, core_ids=list(range(N_CORES)))
    return _gather(res.results)
